# revision 8
# baseline (speedup 1.0000x reference)
"""HQQ+SVD linear kernel for Trainium2, 8-way tensor-parallel (column parallel).

y[b,s,o] = sum_i x[b,s,i] * W_f[o,i] + bias[o]
W_f = (W_q - zp)*scale  (per-group dequant)  + svd_up @ svd_down

Sharding: out-features dim (4096) split across 8 cores (512 each).
x is replicated; W_q/scale/zp/svd_up/bias sharded; svd_down replicated.

Per-core device program:
  1. W-prep: load W_q shard [512,4096] i32, dequant on DVE with per-(o,group)
     scale/zero-point, add low-rank svd correction via PE matmuls, then
     PE-transpose to W_fT [4096,512] resident in SBUF (8 MiB).
  2. Stream x in 64 slabs of [128 tok, 4096]: PE-transpose each slab to
     xT tiles [128 i, 128 t] (float32r transposes), then accumulate
     psum[t,o] over 32 k-tiles with float32r matmuls (full PE rate),
     add bias on DVE, DMA out.
"""

import os
import sys

sys.path.insert(0, "/opt/trn_rl_repo")

import numpy as np

import concourse.bass as bass
import concourse.mybir as mybir
from concourse import bacc
from concourse.masks import make_identity
from concourse.tile import TileContext
from concourse.bass_utils import run_bass_kernel_spmd

OUT, IN, RANK, NG, GS = 4096, 4096, 32, 32, 128
B, S = 4, 2048
T = B * S  # 8192 tokens
N_CORES = 8
OSH = OUT // N_CORES  # 512 out features per core

P = 128
N_OT = OSH // P  # 4 o-tiles per core
N_IT = IN // P  # 32 k-tiles
N_TT = T // P  # 64 token slabs
F32 = mybir.dt.float32
F32R = mybir.dt.float32r
I32 = mybir.dt.int32

MM_DT = os.environ.get("KERNEL_MM_DT", "f32r")  # f32r | f32
MM_TILE_DT = F32R if MM_DT == "f32r" else F32


def build(nc: bass.Bass):
    x = nc.dram_tensor("x", [T, IN], F32, kind="ExternalInput")
    wq = nc.dram_tensor("wq", [OSH, IN], I32, kind="ExternalInput")
    scale = nc.dram_tensor("scale", [OSH, NG], F32, kind="ExternalInput")
    zp = nc.dram_tensor("zp", [OSH, NG], F32, kind="ExternalInput")
    svd_down = nc.dram_tensor("svd_down", [RANK, IN], F32, kind="ExternalInput")
    svd_upT = nc.dram_tensor("svd_upT", [RANK, OSH], F32, kind="ExternalInput")
    bias = nc.dram_tensor("bias", [1, OSH], F32, kind="ExternalInput")
    y = nc.dram_tensor("y", [T, OSH], F32, kind="ExternalOutput")

    with TileContext(nc) as tc:
        with (
            tc.tile_pool(name="consts", bufs=1) as consts,
            tc.tile_pool(name="wfT", bufs=1) as p_wfT,
        ):
            identity = consts.tile([P, P], F32)
            make_identity(nc, identity)

            # W_fT resident: [128 i-part, 32 it, 512 o] (f32r: rounded by the
            # ACT psum->sbuf copy, as the fp32r matmult verifier requires)
            wfT = p_wfT.tile([P, N_IT, OSH], MM_TILE_DT)

            # ---- constants / small tensors ----
            scale_sb = consts.tile([P, N_OT, NG], F32)
            zp_sb = consts.tile([P, N_OT, NG], F32)
            negzs_sb = consts.tile([P, N_OT, NG], F32)
            nc.sync.dma_start(scale_sb[:], scale.ap().rearrange("(a p) g -> p a g", p=P))
            nc.sync.dma_start(zp_sb[:], zp.ap().rearrange("(a p) g -> p a g", p=P))
            # negzs = -(zp * scale)
            nc.vector.tensor_tensor(
                out=negzs_sb[:], in0=zp_sb[:], in1=scale_sb[:], op=mybir.AluOpType.mult
            )
            nc.vector.tensor_scalar_mul(negzs_sb[:], negzs_sb[:], -1.0)

            svdd_sb = consts.tile([RANK, IN], F32)
            svdu_sb = consts.tile([RANK, OSH], F32)
            bias_sb = consts.tile([1, OSH], F32)
            nc.sync.dma_start(svdd_sb[:], svd_down.ap())
            nc.sync.dma_start(svdu_sb[:], svd_upT.ap())
            nc.sync.dma_start(bias_sb[:], bias.ap())

            ones_sb = consts.tile([1, P], F32)
            nc.vector.memset(ones_sb[:], 1.0)
            bias_bc = consts.tile([P, OSH], F32)

            # ---- W prep ----
            with (
                tc.tile_pool(name="wq_sb", bufs=2) as p_wq,
                tc.tile_pool(name="wf_sb", bufs=2) as p_wf,
                tc.tile_pool(name="ps_svd", bufs=2, space="PSUM") as p_svd,
                tc.tile_pool(name="ps_wt", bufs=2, space="PSUM") as p_wt,
            ):
                # broadcast bias to 128 partitions via ones-matmul
                ps_b = p_svd.tile([P, OSH], F32)
                nc.tensor.matmul(ps_b[:], ones_sb[:], bias_sb[:], start=True, stop=True)
                nc.scalar.copy(bias_bc[:], ps_b[:])

                for ot in range(N_OT):
                    wq_t = p_wq.tile([P, IN], I32, tag="wq")
                    nc.sync.dma_start(wq_t[:], wq.ap()[ot * P : (ot + 1) * P, :])
                    wf_t = p_wf.tile([P, IN], F32, tag="wf")
                    # dequant per group: wf = wq * scale + (-zp*scale)
                    for g in range(NG):
                        nc.vector.tensor_scalar(
                            out=wf_t[:, g * GS : (g + 1) * GS],
                            in0=wq_t[:, g * GS : (g + 1) * GS],
                            scalar1=scale_sb[:, ot, g : g + 1],
                            scalar2=negzs_sb[:, ot, g : g + 1],
                            op0=mybir.AluOpType.mult,
                            op1=mybir.AluOpType.add,
                        )
                    # svd correction: wf[o, i] += svd_up@svd_down [o-tile, :]
                    for ic in range(IN // 512):
                        ps = p_svd.tile([P, 512], F32, tag="svd")
                        nc.tensor.matmul(
                            ps[:],
                            svdu_sb[:, ot * P : (ot + 1) * P],
                            svdd_sb[:, ic * 512 : (ic + 1) * 512],
                            start=True,
                            stop=True,
                        )
                        nc.vector.tensor_tensor(
                            out=wf_t[:, ic * 512 : (ic + 1) * 512],
                            in0=wf_t[:, ic * 512 : (ic + 1) * 512],
                            in1=ps[:],
                            op=mybir.AluOpType.add,
                        )
                    # transpose wf [o-tile, i] -> wfT [i, o-tile]
                    for itg in range(N_IT // 4):
                        ps_t = p_wt.tile([P, 512], F32, tag="wt")
                        for j in range(4):
                            it = itg * 4 + j
                            nc.tensor.transpose(
                                ps_t[:, j * P : (j + 1) * P],
                                wf_t[:, it * P : (it + 1) * P],
                                identity[:],
                            )
                        nc.scalar.copy(
                            wfT[:, itg * 4 : itg * 4 + 4, ot * P : (ot + 1) * P],
                            ps_t[:].rearrange("p (a o) -> p a o", a=4),
                        )

            # ---- main loop over token slabs ----
            with (
                tc.tile_pool(name="xs", bufs=3) as p_xs,
                tc.tile_pool(name="xt", bufs=12) as p_xt,
                tc.tile_pool(name="ysb", bufs=3) as p_y,
                tc.tile_pool(name="ps_xt", bufs=2, space="PSUM") as p_psxt,
                tc.tile_pool(name="ps_y", bufs=2, space="PSUM") as p_psy,
            ):
                for tt in range(N_TT):
                    xs = p_xs.tile([P, IN], F32, tag="xs")
                    nc.sync.dma_start(xs[:], x.ap()[tt * P : (tt + 1) * P, :])
                    xt_tiles = []
                    for itg in range(N_IT // 4):
                        ps_t = p_psxt.tile([P, 512], F32, tag="xtp")
                        for j in range(4):
                            it = itg * 4 + j
                            nc.tensor.transpose(
                                ps_t[:, j * P : (j + 1) * P],
                                xs[:, it * P : (it + 1) * P],
                                identity[:],
                            )
                        xtg = p_xt.tile([P, 4, P], MM_TILE_DT, tag="xtg")
                        nc.scalar.copy(
                            xtg[:], ps_t[:].rearrange("p (a t) -> p a t", a=4)
                        )
                        xt_tiles.append(xtg)

                    ps_y = p_psy.tile([P, OSH], F32, tag="y")
                    for it in range(N_IT):
                        nc.tensor.matmul(
                            ps_y[:],
                            xt_tiles[it // 4][:, it % 4, :],
                            wfT[:, it, :],
                            start=(it == 0),
                            stop=(it == N_IT - 1),
                        )
                    y_sb = p_y.tile([P, OSH], F32, tag="ysb")
                    nc.vector.tensor_tensor(
                        out=y_sb[:], in0=ps_y[:], in1=bias_bc[:], op=mybir.AluOpType.add
                    )
                    nc.sync.dma_start(y.ap()[tt * P : (tt + 1) * P, :], y_sb[:])
    return nc


_NC_CACHE = None


def _get_nc():
    global _NC_CACHE
    if _NC_CACHE is None:
        nc = bacc.Bacc(None, target_bir_lowering=False)
        build(nc)
        nc.compile()
        _NC_CACHE = nc
    return _NC_CACHE


def _in_maps(x, W_q, svd_up, svd_down, scale, zero_point, bias):
    x2 = np.ascontiguousarray(np.asarray(x, dtype=np.float32).reshape(T, IN))
    maps = []
    for c in range(N_CORES):
        sl = slice(c * OSH, (c + 1) * OSH)
        maps.append(
            {
                "x": x2,
                "wq": np.ascontiguousarray(
                    np.asarray(W_q, dtype=np.int32)[sl].reshape(OSH, IN)
                ),
                "scale": np.ascontiguousarray(np.asarray(scale, dtype=np.float32)[sl]),
                "zp": np.ascontiguousarray(
                    np.asarray(zero_point, dtype=np.float32)[sl]
                ),
                "svd_down": np.ascontiguousarray(
                    np.asarray(svd_down, dtype=np.float32)
                ),
                "svd_upT": np.ascontiguousarray(
                    np.asarray(svd_up, dtype=np.float32)[sl].T
                ),
                "bias": np.ascontiguousarray(
                    np.asarray(bias, dtype=np.float32)[sl].reshape(1, OSH)
                ),
            }
        )
    return maps


def _run(in_maps, **kw):
    nc = _get_nc()
    return run_bass_kernel_spmd(nc, in_maps, core_ids=list(range(N_CORES)), **kw)


def kernel(x, W_q, svd_up, svd_down, scale, zero_point, bias):
    res = _run(_in_maps(x, W_q, svd_up, svd_down, scale, zero_point, bias))
    y = np.concatenate([res.results[c]["y"] for c in range(N_CORES)], axis=1)
    return y.reshape(B, S, OUT)
